# revision 1
# baseline (speedup 1.0000x reference)
"""Trainium2 Bass kernel for the JaCDE dense-MLP vector-field problem.

Math restructuring: the reference materializes d_outer[b,i,j] = dtanh[b,i] *
wout[i,j] * drelu[b,j] and contracts it (O(B*H^3)).  Algebraically the whole
computation is 9 applications of the per-sample linear operator
    M v = dtanh ∘ (wout @ (drelu ∘ (wh @ v)))
applied to jx = dtanh ∘ (wout @ (drelu ∘ (wx @ xdot))), with
    h_dot = sum_{k=0..8} M^k jx
which is O(B*H^2) matmuls.  All activations live transposed [H, B_local] so
batch is the matmul free dim (N=256) and H splits into 2 partition tiles.

Sharding: pure data parallel, batch 2048 -> 8 cores x 256.
"""

import numpy as np

import concourse.tile as tile
from concourse import bacc, mybir
from concourse.bass_utils import run_bass_kernel_spmd

B, H, IN = 2048, 256, 64
K_TERMS = 8
N_CORES = 8
BL = B // N_CORES  # 256 batch rows per core
HH = H // 2  # 128, H partition halves

f32 = mybir.dt.float32
# float32r: PE reads fp32 operands through the fast (1 cycle/row for N>=256)
# path with mantissa rounding; measured max-rel-err ~1.6e-4 on HW, same
# envelope as the plain fp32 PE path which runs 4x slower.
MM_DT = mybir.dt.float32r
N_WARMUP_MM = 16  # ~3us of PE work to lift the HAM clock gate before real MMs

_ALU = mybir.AluOpType
_ACT = mybir.ActivationFunctionType


def _build(repeat=1, loop=0):
    nc = bacc.Bacc(None, target_bir_lowering=False)

    # Per-core inputs (already transposed / sharded by the host wrapper).
    hT = nc.dram_tensor("hT", [H, BL], f32, kind="ExternalInput")
    xT = nc.dram_tensor("xT", [IN, BL], f32, kind="ExternalInput")
    xdT = nc.dram_tensor("xdT", [IN, BL], MM_DT, kind="ExternalInput")
    wxT = nc.dram_tensor("wxT", [IN, H], f32, kind="ExternalInput")
    whT = nc.dram_tensor("whT", [H, H], f32, kind="ExternalInput")
    woT = nc.dram_tensor("woT", [H, H], MM_DT, kind="ExternalInput")
    b0c = nc.dram_tensor("b0c", [HH, 2], f32, kind="ExternalInput")
    b1c = nc.dram_tensor("b1c", [HH, 2], f32, kind="ExternalInput")
    hdT = nc.dram_tensor("hdT", [H, BL], f32, kind="ExternalOutput")

    with tile.TileContext(nc) as tc:
        with (
            tc.tile_pool(name="wpool", bufs=1) as wpool,
            tc.tile_pool(name="apool", bufs=1) as apool,
            tc.tile_pool(name="rot", bufs=4) as rot,
            tc.tile_pool(name="tgp", bufs=3) as tgp,
            tc.tile_pool(name="ps", bufs=8, space="PSUM") as ps,
        ):
            # ---- weights / inputs to SBUF ----
            whF = [wpool.tile([HH, H], f32, tag=f"whF{k}", name=f"whF{k}") for k in range(2)]
            wh_sb = [wpool.tile([HH, H], MM_DT, tag=f"wh{k}", name=f"wh{k}") for k in range(2)]
            wo_sb = [wpool.tile([HH, H], MM_DT, tag=f"wo{k}", name=f"wo{k}") for k in range(2)]
            wxF = wpool.tile([IN, H], f32, tag="wxF")
            wx_sb = wpool.tile([IN, H], MM_DT, tag="wx")
            h_sb = [wpool.tile([HH, BL], f32, tag=f"h{k}", name=f"h{k}") for k in range(2)]
            x_sb = wpool.tile([IN, BL], f32, tag="x")
            xd_sb = wpool.tile([IN, BL], MM_DT, tag="xd")
            b0_sb = wpool.tile([HH, 2], f32, tag="b0")
            b1_sb = wpool.tile([HH, 2], f32, tag="b1")
            for k in range(2):
                nc.sync.dma_start(whF[k][:], whT[k * HH : (k + 1) * HH, :])
                nc.sync.dma_start(wo_sb[k][:], woT[k * HH : (k + 1) * HH, :])
                nc.sync.dma_start(h_sb[k][:], hT[k * HH : (k + 1) * HH, :])
                # rounded copies for the fast f32r loop matmuls
                nc.vector.tensor_copy(wh_sb[k][:], whF[k][:])
            nc.sync.dma_start(wxF[:], wxT[:])
            nc.vector.tensor_copy(wx_sb[:], wxF[:])
            nc.sync.dma_start(x_sb[:], xT[:])
            nc.sync.dma_start(xd_sb[:], xdT[:])
            nc.sync.dma_start(b0_sb[:], b0c[:])
            nc.sync.dma_start(b1_sb[:], b1c[:])

            # ---- PE warmup: dummy matmuls with no data deps so the HAM
            # clock gate opens while the input DMAs are still in flight ----
            if N_WARMUP_MM:
                wu_w = wpool.tile([HH, HH], MM_DT, tag="wu_w")
                wu_v = wpool.tile([HH, BL], MM_DT, tag="wu_v")
                nc.vector.memset(wu_w[:].bitcast(f32), 0.0)
                nc.vector.memset(wu_v[:].bitcast(f32), 0.0)
                wu_p = ps.tile([HH, BL], f32, tag="ps")
                for _ in range(N_WARMUP_MM):
                    nc.tensor.matmul(
                        wu_p[:], wu_w[:], wu_v[:], start=True, stop=True
                    )

            def mm_H(psum, w_pair, rhs_pair, m, extra=None):
                """psum[m] += W @ rhs for a [H,H] weight given as 2 k-tiles."""
                first = extra is None
                if extra is not None:
                    nc.tensor.matmul(
                        psum[:], extra[0][:, m * HH : (m + 1) * HH], extra[1][:],
                        start=True, stop=False,
                    )
                nc.tensor.matmul(
                    psum[:], w_pair[0][:, m * HH : (m + 1) * HH], rhs_pair[0][:],
                    start=first, stop=False,
                )
                nc.tensor.matmul(
                    psum[:], w_pair[1][:, m * HH : (m + 1) * HH], rhs_pair[1][:],
                    start=False, stop=True,
                )

            import contextlib
            loop_cm = tc.For_i(0, loop, 1) if loop else contextlib.nullcontext()
            with loop_cm:
             for _rep in range(repeat):
              # ---- phase 1: l1 = wx@xT + wh@hT + b0; relu & keep l1 for gate ----
              l1_sb = [apool.tile([HH, BL], f32, tag=f"l1_{m}", name=f"l1_{m}") for m in range(2)]
              relu_sb = [apool.tile([HH, BL], MM_DT, tag=f"relu{m}", name=f"relu{m}") for m in range(2)]
              for m in range(2):
                  p = ps.tile([HH, BL], f32, tag="ps")
                  mm_H(p, whF, h_sb, m, extra=(wxF, x_sb))
                  nc.scalar.activation(
                      l1_sb[m][:], p[:], _ACT.Identity, bias=b0_sb[:, m : m + 1]
                  )
                  nc.scalar.activation(
                      relu_sb[m][:], p[:], _ACT.Relu, bias=b0_sb[:, m : m + 1]
                  )

              # ---- phase 2: lout = wout@relu + b1; tanh; dtanh = 1-tanh^2 ----
              dtanh_sb = [apool.tile([HH, BL], f32, tag=f"dt{m}", name=f"dt{m}") for m in range(2)]
              for m in range(2):
                  p = ps.tile([HH, BL], f32, tag="ps")
                  mm_H(p, wo_sb, relu_sb, m)
                  tanh = apool.tile([HH, BL], f32, tag=f"tanh{m}")
                  nc.scalar.activation(
                      tanh[:], p[:], _ACT.Tanh, bias=b1_sb[:, m : m + 1]
                  )
                  nc.vector.tensor_mul(dtanh_sb[m][:], tanh[:], tanh[:])
                  nc.vector.tensor_scalar(
                      dtanh_sb[m][:], dtanh_sb[m][:], -1.0, 1.0, _ALU.mult, _ALU.add
                  )

              # ---- phase 3: jx = dtanh ∘ (wout @ (drelu ∘ (wx @ xdot))) ----
              g_sb = [tgp.tile([HH, BL], MM_DT, tag=f"tg{m}", name=f"g{m}") for m in range(2)]
              for m in range(2):
                  p = ps.tile([HH, BL], f32, tag="ps")
                  nc.tensor.matmul(
                      p[:], wx_sb[:, m * HH : (m + 1) * HH], xd_sb[:],
                      start=True, stop=True,
                  )
                  # g = (l1 > 0) * u   -- fused relu-derivative gate
                  nc.vector.scalar_tensor_tensor(
                      g_sb[m][:], l1_sb[m][:], 0.0, p[:], _ALU.is_gt, _ALU.mult
                  )
              curr = [None, None]
              hdot = [apool.tile([HH, BL], f32, tag=f"hd{m}", name=f"hd{m}") for m in range(2)]
              for m in range(2):
                  p = ps.tile([HH, BL], f32, tag="ps")
                  mm_H(p, wo_sb, g_sb, m)
                  c = rot.tile([HH, BL], MM_DT, tag=f"curr{m}", name=f"curr{m}")
                  nc.vector.tensor_mul(c[:], dtanh_sb[m][:], p[:])
                  curr[m] = c
                  # h_dot starts as jx (ACT engine, off the critical path)
                  nc.scalar.copy(hdot[m][:], c[:].bitcast(f32))

              # ---- phase 4: 8 iterations of curr <- M curr; hdot += curr ----
              def mm_H_kouter(psums, w_pair, rhs_pair):
                  # k-outer order: the first two matmuls only need rhs_pair[0],
                  # so they start as soon as the k=0 half of the rhs lands.
                  for k in range(2):
                      for m in range(2):
                          nc.tensor.matmul(
                              psums[m][:],
                              w_pair[k][:, m * HH : (m + 1) * HH],
                              rhs_pair[k][:],
                              start=(k == 0), stop=(k == 1),
                          )

              for _ in range(K_TERMS):
                  tg = [tgp.tile([HH, BL], MM_DT, tag=f"tg{m}", name=f"tg{m}") for m in range(2)]
                  pt = [ps.tile([HH, BL], f32, tag="ps", name=f"pt{m}") for m in range(2)]
                  mm_H_kouter(pt, wh_sb, curr)
                  for m in range(2):
                      nc.vector.scalar_tensor_tensor(
                          tg[m][:], l1_sb[m][:], 0.0, pt[m][:], _ALU.is_gt, _ALU.mult
                      )
                  newc = [None, None]
                  pso = [ps.tile([HH, BL], f32, tag="ps", name=f"pso{m}") for m in range(2)]
                  mm_H_kouter(pso, wo_sb, tg)
                  for m in range(2):
                      c = rot.tile([HH, BL], MM_DT, tag=f"curr{m}", name=f"curr{m}")
                      nc.vector.tensor_mul(c[:], dtanh_sb[m][:], pso[m][:])
                      newc[m] = c
                      # accumulate on GpSimd so DVE stays on the critical path
                      nc.gpsimd.tensor_add(
                          hdot[m][:], hdot[m][:], c[:].bitcast(f32)
                      )
                  curr = newc

              for m in range(2):
                  nc.sync.dma_start(hdT[m * HH : (m + 1) * HH, :], hdot[m][:])

    nc.compile()
    return nc


_NC = {}


def _get_nc(repeat=1, loop=0):
    key = (repeat, loop)
    if key not in _NC:
        _NC[key] = _build(repeat, loop)
    return _NC[key]


def kernel(h, x, xdot, wx, wh, wout, b0, b1):
    h = np.asarray(h, np.float32)
    x = np.asarray(x, np.float32)
    xdot = np.asarray(xdot, np.float32)
    wx = np.asarray(wx, np.float32)
    wh = np.asarray(wh, np.float32)
    wout = np.asarray(wout, np.float32)
    b0 = np.asarray(b0, np.float32)
    b1 = np.asarray(b1, np.float32)

    whT = np.ascontiguousarray(wh.T)
    woT = np.ascontiguousarray(wout.T)
    wxT = np.ascontiguousarray(wx.T)
    b0c = np.ascontiguousarray(np.stack([b0[:HH], b0[HH:]], axis=1))
    b1c = np.ascontiguousarray(np.stack([b1[:HH], b1[HH:]], axis=1))

    in_maps = []
    for i in range(N_CORES):
        sl = slice(i * BL, (i + 1) * BL)
        in_maps.append(
            {
                "hT": np.ascontiguousarray(h[sl].T),
                "xT": np.ascontiguousarray(x[sl].T),
                "xdT": np.ascontiguousarray(xdot[sl].T),
                "wxT": wxT,
                "whT": whT,
                "woT": woT,
                "b0c": b0c,
                "b1c": b1c,
            }
        )

    res = run_bass_kernel_spmd(_get_nc(), in_maps, core_ids=list(range(N_CORES)))
    out = np.empty((B, H), np.float32)
    for i in range(N_CORES):
        out[i * BL : (i + 1) * BL] = res.results[i]["hdT"].T
    return out



# revision 10
# speedup vs baseline: 2.4790x; 2.4790x over previous
"""Trainium2 Bass kernel for the JaCDE dense-MLP vector-field problem.

Math: h_dot = sum_{k=0..8} M^k jx with the per-sample linear operator
    M v = dtanh ** (wout @ (drelu ** (wh @ v)))
    jx  = dtanh ** (wout @ (drelu ** (wx @ xdot)))
(** = elementwise), all activations transposed [H, B_local].

Structure (per core, BL=256 batch):
 - Dual-stream software pipeline: batch split into 2 streams of 128 cols;
   loop matmuls are bf16 [K=128 -> M=128, N=128] (1 cycle/row at any N),
   stages interleaved A B A B so PE/DVE/ACT/Pool all stay busy and the
   ~100ns cross-engine sem latency is hidden.
 - GPSIMD (Pool) cannot touch PSUM on HW, so PSUM consumers split:
   relu-gates on DVE (cheap tensor ops on PSUM), PSUM->SBUF evacuation
   of the wout product on ACT (copy activation), dtanh-mul + hdot
   accumulation on Pool in SBUF.
 - l1 (relu gate pre-activation) lives in PSUM all loop, stream-major
   [128 part, m0|m1 cols]; b0 is folded in via a K=1 ones-vector matmul.
 - Phase 1 (l1 = wx@x + wh@h) uses a bf16 hi/lo split
   (A@B ~ A1B1 + A1B2 + A2B1): fp32-class accuracy (~4e-6 on l1, keeps
   the hard relu gate exact) at bf16 matmul speed.
 - Loop weights/activations are single bf16 (measured end-to-end rel err
   ~6.1e-3 vs the fp32 reference, tolerance 2e-2).
 - DMA: input DMAs spread over SP/ACT queues, big phase-1 blobs first;
   ACT table preloaded via a dummy tanh at t~0; PE p-state warmed with
   dummy matmuls during the DMA wait.

Sharding: pure data parallel, batch 2048 -> 8 cores x 256.
"""

import numpy as np

import concourse.tile as tile
from concourse import bacc, mybir
from concourse.bass_utils import run_bass_kernel_spmd

B, H, IN = 2048, 256, 64
K_TERMS = 8
N_CORES = 8
BL = B // N_CORES  # 256 batch rows per core

f32 = mybir.dt.float32
bf16 = mybir.dt.bfloat16
_ALU = mybir.AluOpType
_ACT = mybir.ActivationFunctionType

N_WARMUP_MM = 12  # ~2.6us of PE work to lift the p-state before phase 1


def _build(repeat=1, loop=0):
    nc = bacc.Bacc(None, target_bir_lowering=False)

    # hwk{k} = [whT_hi[k] (256) | whT_lo[k] (256) | hT_hi[k] (256) | hT_lo[k]]
    d_hw = [
        nc.dram_tensor(f"hwk{k}", [128, 1024], bf16, kind="ExternalInput")
        for k in (0, 1)
    ]
    # xw = [wxT_hi (256) | wxT_lo | xT_hi | xT_lo]   (64 partitions)
    d_xw = nc.dram_tensor("xw", [64, 1024], bf16, kind="ExternalInput")
    # wo = [woT_hi[k0] (256) | woT_hi[k1]]
    d_wo = nc.dram_tensor("wo", [128, 512], bf16, kind="ExternalInput")
    d_xd = nc.dram_tensor("xd", [64, 256], bf16, kind="ExternalInput")
    # bias row-vectors: [b0 m0 (128) | b0 m1 | ones (128) | pad]
    d_bias = nc.dram_tensor("bias", [1, 512], bf16, kind="ExternalInput")
    # cst cols: 4,5 = b1 halves (f32, ACT tanh bias)
    d_c = nc.dram_tensor("cst", [128, 6], f32, kind="ExternalInput")
    hdT = nc.dram_tensor("hdT", [H, BL], f32, kind="ExternalOutput")

    with tile.TileContext(nc) as tc:
        with (
            tc.tile_pool(name="wpool", bufs=1) as wpool,
            tc.tile_pool(name="apool", bufs=1) as apool,
            tc.tile_pool(name="rot", bufs=2) as rot,
            tc.tile_pool(name="ps", bufs=1, space="PSUM") as ps,
        ):
            HW = [wpool.tile([128, 1024], bf16, tag=f"hw{k}", name=f"hw{k}") for k in (0, 1)]
            XW = wpool.tile([64, 1024], bf16, tag="xw")
            WOT = wpool.tile([128, 512], bf16, tag="wo")
            XDT = wpool.tile([64, 256], bf16, tag="xd")
            BIA = wpool.tile([1, 512], bf16, tag="bias")
            CST = wpool.tile([128, 6], f32, tag="cst")
            wu_w = wpool.tile([128, 128], bf16, tag="wu_w")
            wu_v = wpool.tile([128, 256], bf16, tag="wu_v")
            dmy = wpool.tile([1, 1], f32, tag="dmy")

            # DVE: memsets for warmup/dummy tiles
            nc.vector.memset(wu_w[:].bitcast(f32), 0.0)
            nc.vector.memset(wu_v[:].bitcast(f32), 0.0)
            nc.vector.memset(dmy[:], 0.0)

            # SP: phase-1 blobs first (k0, k1), then wx/x
            nc.sync.dma_start(HW[0][:], d_hw[0][:])
            nc.sync.dma_start(HW[1][:], d_hw[1][:])
            nc.sync.dma_start(XW[:], d_xw[:])

            # ACT: small consts, dummy tanh (forces the activation-table
            # load at t~0), then the non-critical DMAs
            nc.scalar.dma_start(BIA[:], d_bias[:])
            nc.scalar.dma_start(CST[:], d_c[:])
            nc.scalar.activation(dmy[:], dmy[:], _ACT.Tanh, bias=dmy[:])
            nc.scalar.dma_start(WOT[:], d_wo[:])
            nc.scalar.dma_start(XDT[:], d_xd[:])

            # PE p-state warmup (no data deps; runs during the DMA wait)
            wu_p = ps.tile([128, 256], f32, tag="lo", name="wu_p")
            for _ in range(N_WARMUP_MM):
                nc.tensor.matmul(wu_p[:], wu_w[:], wu_v[:], start=True, stop=True)

            import contextlib

            loop_cm = tc.For_i(0, loop, 1) if loop else contextlib.nullcontext()
            with loop_cm:
             for _rep in range(repeat):
              # ---- phase 1: l1 = wx@x + wh@h + b0, bf16 hi/lo, exact gate.
              # L1S[s]: stream-major PSUM [128, m0|m1], resident all loop.
              L1S = [ps.tile([128, 256], f32, tag=f"l1{s}", name=f"l1{s}") for s in (0, 1)]
              for m in (0, 1):
                  for s in (0, 1):
                      dst = L1S[s][:, m * 128 : (m + 1) * 128]
                      terms = []
                      for k in (0, 1):
                          w_hi = HW[k][:, m * 128 : (m + 1) * 128]
                          w_lo = HW[k][:, 256 + m * 128 : 256 + (m + 1) * 128]
                          h_hi = HW[k][:, 512 + s * 128 : 512 + (s + 1) * 128]
                          h_lo = HW[k][:, 768 + s * 128 : 768 + (s + 1) * 128]
                          terms += [(w_hi, h_hi), (w_hi, h_lo), (w_lo, h_hi)]
                      wx_hi = XW[:, m * 128 : (m + 1) * 128]
                      wx_lo = XW[:, 256 + m * 128 : 256 + (m + 1) * 128]
                      x_hi = XW[:, 512 + s * 128 : 512 + (s + 1) * 128]
                      x_lo = XW[:, 768 + s * 128 : 768 + (s + 1) * 128]
                      terms += [(wx_hi, x_hi), (wx_hi, x_lo), (wx_lo, x_hi)]
                      # += b0[m] (outer product with ones), closes the group
                      terms += [(BIA[:, m * 128 : (m + 1) * 128], BIA[:, 256:384])]
                      for i, (lhsT, rhs) in enumerate(terms):
                          nc.tensor.matmul(
                              dst, lhsT, rhs,
                              start=(i == 0), stop=(i == len(terms) - 1),
                          )

              # ---- phase 3a: u = wx@xdot (PSUM tag shared with loop W) ----
              US = [ps.tile([128, 256], f32, tag=f"w{s}", name=f"u{s}") for s in (0, 1)]
              for s in (0, 1):
                  for m in (0, 1):
                      nc.tensor.matmul(
                          US[s][:, m * 128 : (m + 1) * 128],
                          XW[:, m * 128 : (m + 1) * 128],
                          XDT[:, s * 128 : (s + 1) * 128],
                          start=True, stop=True,
                      )

              # ---- relu on DVE (PSUM read): relu[k] m-major bf16 ----
              relu = [apool.tile([128, 256], bf16, tag=f"relu{k}", name=f"relu{k}") for k in (0, 1)]
              for k in (0, 1):
                  for s in (0, 1):
                      nc.vector.tensor_scalar_max(
                          relu[k][:, s * 128 : (s + 1) * 128],
                          L1S[s][:, k * 128 : (k + 1) * 128], 0.0,
                      )

              # ---- phase 2: lout mms (m-major); tanh on ACT ----
              LOPS = ps.tile([128, 512], f32, tag="lo", name="lo")
              LO = [LOPS[:, 0:256], LOPS[:, 256:512]]
              for m in (0, 1):
                  for k in (0, 1):
                      nc.tensor.matmul(
                          LO[m], WOT[:, k * 256 + m * 128 : k * 256 + (m + 1) * 128],
                          relu[k][:], start=(k == 0), stop=(k == 1),
                      )
              tanh_t = [apool.tile([128, 256], f32, tag=f"tanh{m}", name=f"tanh{m}") for m in (0, 1)]
              for m in (0, 1):
                  nc.scalar.activation(
                      tanh_t[m][:], LO[m], _ACT.Tanh, bias=CST[:, 4 + m : 5 + m]
                  )

              # ---- gate mask ds = (l1 > 0) as 0/1, once, SBUF ----
              ds = [apool.tile([128, 256], bf16, tag=f"ds{s}", name=f"ds{s}") for s in (0, 1)]
              for s in (0, 1):
                  nc.vector.tensor_scalar(
                      ds[s][:], L1S[s][:], 0.0, None, _ALU.is_gt
                  )

              # ---- phase 3b: g = ds * u on DVE (stream-major) ----
              gs = [apool.tile([128, 256], bf16, tag=f"g{s}", name=f"g{s}") for s in (0, 1)]
              for s in (0, 1):
                  nc.vector.tensor_mul(gs[s][:], ds[s][:], US[s][:])

              # ---- dtanh, stream-major f32, on Pool (SBUF only) ----
              sqs = [apool.tile([128, 256], f32, tag=f"sq{s}", name=f"sq{s}") for s in (0, 1)]
              dts = [apool.tile([128, 256], f32, tag=f"dt{s}", name=f"dt{s}") for s in (0, 1)]
              for s in (0, 1):
                  for m in (0, 1):
                      tslice = tanh_t[m][:, s * 128 : (s + 1) * 128]
                      nc.gpsimd.tensor_mul(
                          sqs[s][:, m * 128 : (m + 1) * 128], tslice, tslice
                      )
                      nc.gpsimd.tensor_scalar(
                          dts[s][:, m * 128 : (m + 1) * 128],
                          sqs[s][:, m * 128 : (m + 1) * 128],
                          -1.0, 1.0, _ALU.mult, _ALU.add,
                      )

              hdot = [apool.tile([128, 256], f32, tag=f"hdot{s}", name=f"hdot{s}") for s in (0, 1)]
              cur = {}
              cur_tg = {}

              def stage_wo_mul(s, it):
                  """wout mms (rhs = tg or g), ACT-evacuate PSUM, Pool mul."""
                  W = ps.tile([128, 256], f32, tag=f"w{s}", name=f"W{s}_{it}")
                  src = gs[s] if it == 0 else cur_tg[s]
                  for m in (0, 1):
                      for k in (0, 1):
                          nc.tensor.matmul(
                              W[:, m * 128 : (m + 1) * 128],
                              WOT[:, k * 256 + m * 128 : k * 256 + (m + 1) * 128],
                              src[:, k * 128 : (k + 1) * 128],
                              start=(k == 0), stop=(k == 1),
                          )
                  c = rot.tile([128, 256], bf16, tag=f"c{s}", name=f"c{s}")
                  nc.vector.tensor_mul(c[:], dts[s][:], W[:])
                  return c

              def stage_wh_gate(s, it):
                  """wh mms on cur[s], then tg = (l1 > 0) * V on DVE."""
                  V = ps.tile([128, 256], f32, tag=f"v{s}", name=f"V{s}_{it}")
                  for m in (0, 1):
                      for k in (0, 1):
                          nc.tensor.matmul(
                              V[:, m * 128 : (m + 1) * 128],
                              HW[k][:, m * 128 : (m + 1) * 128],
                              cur[s][:, k * 128 : (k + 1) * 128],
                              start=(k == 0), stop=(k == 1),
                          )
                  t = rot.tile([128, 256], bf16, tag=f"tg{s}", name=f"tg{s}")
                  nc.vector.tensor_mul(t[:], ds[s][:], V[:])
                  return t

              # phase 3c: jx per stream; hdot initialized from it on DVE
              for s in (0, 1):
                  c0 = stage_wo_mul(s, 0)
                  cur[s] = c0
                  nc.gpsimd.tensor_copy(hdot[s][:], c0[:])

              for it in range(1, K_TERMS + 1):
                  for s in (0, 1):
                      cur_tg[s] = stage_wh_gate(s, it)
                  for s in (0, 1):
                      newc = stage_wo_mul(s, it)
                      nc.gpsimd.tensor_add(hdot[s][:], hdot[s][:], newc[:])
                      cur[s] = newc

              # ---- output: 4 DMAs, stream-0 pair on SP, stream-1 on ACT ----
              qs = {0: nc.sync, 1: nc.scalar}
              for s in (0, 1):
                  for m in (0, 1):
                      qs[s].dma_start(
                          hdT[m * 128 : (m + 1) * 128, s * 128 : (s + 1) * 128],
                          hdot[s][:, m * 128 : (m + 1) * 128],
                      )

    nc.compile()
    return nc


_NC = {}


def _get_nc(repeat=1, loop=0):
    key = (repeat, loop)
    if key not in _NC:
        _NC[key] = _build(repeat, loop)
    return _NC[key]


def _bf(a):
    """Round f32 -> bf16 (RNE) on host, returns ml_dtypes.bfloat16 array."""
    import ml_dtypes

    u = np.ascontiguousarray(np.asarray(a, np.float32)).view(np.uint32)
    rounded = u + np.uint32(0x7FFF) + ((u >> np.uint32(16)) & np.uint32(1))
    return (rounded >> np.uint32(16)).astype(np.uint16).view(ml_dtypes.bfloat16)


def _hilo(a):
    a = np.asarray(a, np.float32)
    hi = _bf(a)
    lo = _bf(a - hi.astype(np.float32))
    return hi, lo


def make_in_maps(h, x, xdot, wx, wh, wout, b0, b1):
    h = np.asarray(h, np.float32)
    x = np.asarray(x, np.float32)
    xdot = np.asarray(xdot, np.float32)
    wx = np.asarray(wx, np.float32)
    wh = np.asarray(wh, np.float32)
    wout = np.asarray(wout, np.float32)
    b0 = np.asarray(b0, np.float32)
    b1 = np.asarray(b1, np.float32)

    whT = np.ascontiguousarray(wh.T)  # [j, i]
    woT = np.ascontiguousarray(wout.T)
    wxT = np.ascontiguousarray(wx.T)  # [IN, H]
    wh_hi, wh_lo = _hilo(whT)
    wx_hi, wx_lo = _hilo(wxT)
    wo_hi = _bf(woT)

    d_wo = np.ascontiguousarray(
        np.concatenate([wo_hi[0:128, :], wo_hi[128:256, :]], axis=1)
    )
    bias = np.zeros((1, 512), np.float32)
    bias[0, 0:256] = b0
    bias[0, 256:384] = 1.0
    d_bias = _bf(bias)
    cst = np.stack(
        [-b0[:128], -b0[128:], b0[:128], b0[128:], b1[:128], b1[128:]], axis=1
    ).astype(np.float32)
    cst = np.ascontiguousarray(cst)

    in_maps = []
    for i in range(N_CORES):
        sl = slice(i * BL, (i + 1) * BL)
        hT = np.ascontiguousarray(h[sl].T)  # [H, BL]
        xT = np.ascontiguousarray(x[sl].T)  # [IN, BL]
        xdT = np.ascontiguousarray(xdot[sl].T)
        h_hi, h_lo = _hilo(hT)
        x_hi, x_lo = _hilo(xT)
        m = {
            "hwk0": np.ascontiguousarray(np.concatenate(
                [wh_hi[0:128], wh_lo[0:128], h_hi[0:128], h_lo[0:128]], axis=1)),
            "hwk1": np.ascontiguousarray(np.concatenate(
                [wh_hi[128:256], wh_lo[128:256], h_hi[128:256], h_lo[128:256]], axis=1)),
            "xw": np.ascontiguousarray(
                np.concatenate([wx_hi, wx_lo, x_hi, x_lo], axis=1)),
            "wo": d_wo,
            "xd": _bf(xdT),
            "bias": d_bias,
            "cst": cst,
        }
        in_maps.append(m)
    return in_maps


def kernel(h, x, xdot, wx, wh, wout, b0, b1):
    in_maps = make_in_maps(h, x, xdot, wx, wh, wout, b0, b1)
    res = run_bass_kernel_spmd(_get_nc(), in_maps, core_ids=list(range(N_CORES)))
    out = np.empty((B, H), np.float32)
    for i in range(N_CORES):
        out[i * BL : (i + 1) * BL] = res.results[i]["hdT"].T
    return out


# revision 11
# speedup vs baseline: 2.5995x; 1.0486x over previous
"""Trainium2 Bass kernel for the JaCDE dense-MLP vector-field problem.

Math: h_dot = sum_{k=0..8} M^k jx with the per-sample linear operator
    M v = dtanh ** (wout @ (drelu ** (wh @ v)))
    jx  = dtanh ** (wout @ (drelu ** (wx @ xdot)))
(** = elementwise), all activations transposed [H, B_local].

Structure (per core, BL=256 batch):
 - Dual-stream software pipeline: batch split into 2 streams of 128 cols;
   loop matmuls are bf16 [K=128 -> M=128, N=128] (1 cycle/row at any N),
   stages interleaved A B A B so PE/DVE/ACT/Pool all stay busy and the
   ~100ns cross-engine sem latency is hidden.
 - GPSIMD (Pool) cannot touch PSUM on HW, so PSUM consumers split:
   relu-gates on DVE (cheap tensor ops on PSUM), PSUM->SBUF evacuation
   of the wout product on ACT (copy activation), dtanh-mul + hdot
   accumulation on Pool in SBUF.
 - l1 (relu gate pre-activation) lives in PSUM all loop, stream-major
   [128 part, m0|m1 cols]; b0 is folded in via a K=1 ones-vector matmul.
 - Phase 1 (l1 = wx@x + wh@h) uses a bf16 hi/lo split
   (A@B ~ A1B1 + A1B2 + A2B1): fp32-class accuracy (~4e-6 on l1, keeps
   the hard relu gate exact) at bf16 matmul speed.
 - Loop weights/activations are single bf16 (measured end-to-end rel err
   ~6.1e-3 vs the fp32 reference, tolerance 2e-2).
 - DMA: input DMAs spread over SP/ACT queues, big phase-1 blobs first;
   ACT table preloaded via a dummy tanh at t~0; PE p-state warmed with
   dummy matmuls during the DMA wait.

Sharding: pure data parallel, batch 2048 -> 8 cores x 256.
"""

import numpy as np

import concourse.tile as tile
from concourse import bacc, mybir
from concourse.bass_utils import run_bass_kernel_spmd

B, H, IN = 2048, 256, 64
K_TERMS = 8
N_CORES = 8
BL = B // N_CORES  # 256 batch rows per core

f32 = mybir.dt.float32
bf16 = mybir.dt.bfloat16
_ALU = mybir.AluOpType
_ACT = mybir.ActivationFunctionType

N_WARMUP_MM = 12  # ~2.6us of PE work to lift the p-state before phase 1


def _build(repeat=1, loop=0):
    nc = bacc.Bacc(None, target_bir_lowering=False)

    # hwk{k} = [whT_hi[k] (256) | whT_lo[k] (256) | hT_hi[k] (256) | hT_lo[k]]
    d_hw = [
        nc.dram_tensor(f"hwk{k}", [128, 1024], bf16, kind="ExternalInput")
        for k in (0, 1)
    ]
    # xw = [wxT_hi (256) | wxT_lo | xT_hi | xT_lo]   (64 partitions)
    d_xw = nc.dram_tensor("xw", [64, 1024], bf16, kind="ExternalInput")
    # wo = [woT_hi[k0] (256) | woT_hi[k1]]
    d_wo = nc.dram_tensor("wo", [128, 512], bf16, kind="ExternalInput")
    d_xd = nc.dram_tensor("xd", [64, 256], bf16, kind="ExternalInput")
    # bias row-vectors: [b0 (256) | ones (256)]
    d_bias = nc.dram_tensor("bias", [1, 512], bf16, kind="ExternalInput")
    # cst cols: 4,5 = b1 halves (f32, ACT tanh bias)
    d_c = nc.dram_tensor("cst", [128, 6], f32, kind="ExternalInput")
    hdT = nc.dram_tensor("hdT", [H, BL], f32, kind="ExternalOutput")

    with tile.TileContext(nc) as tc:
        with (
            tc.tile_pool(name="wpool", bufs=1) as wpool,
            tc.tile_pool(name="apool", bufs=1) as apool,
            tc.tile_pool(name="rot", bufs=2) as rot,
            tc.tile_pool(name="ps", bufs=1, space="PSUM") as ps,
        ):
            HW = [wpool.tile([128, 1024], bf16, tag=f"hw{k}", name=f"hw{k}") for k in (0, 1)]
            XW = wpool.tile([64, 1024], bf16, tag="xw")
            WOT = wpool.tile([128, 512], bf16, tag="wo")
            XDT = wpool.tile([64, 256], bf16, tag="xd")
            BIA = wpool.tile([1, 512], bf16, tag="bias")
            CST = wpool.tile([128, 6], f32, tag="cst")
            wu_w = wpool.tile([128, 128], bf16, tag="wu_w")
            wu_v = wpool.tile([128, 256], bf16, tag="wu_v")
            dmy = wpool.tile([1, 1], f32, tag="dmy")

            # DVE: memsets for warmup/dummy tiles
            nc.vector.memset(wu_w[:].bitcast(f32), 0.0)
            nc.vector.memset(wu_v[:].bitcast(f32), 0.0)
            nc.vector.memset(dmy[:], 0.0)

            # SP: phase-1 blobs first (k0, k1), then wx/x
            nc.sync.dma_start(HW[0][:], d_hw[0][:])
            nc.sync.dma_start(HW[1][:], d_hw[1][:])
            nc.sync.dma_start(XW[:], d_xw[:])

            # ACT: small consts, dummy tanh (forces the activation-table
            # load at t~0), then the non-critical DMAs
            nc.scalar.dma_start(BIA[:], d_bias[:])
            nc.scalar.dma_start(CST[:], d_c[:])
            nc.scalar.activation(dmy[:], dmy[:], _ACT.Tanh, bias=dmy[:])
            nc.scalar.dma_start(WOT[:], d_wo[:])
            nc.scalar.dma_start(XDT[:], d_xd[:])

            # PE p-state warmup (no data deps; runs during the DMA wait)
            wu_p = ps.tile([128, 256], f32, tag="lo0", name="wu_p")
            for _ in range(N_WARMUP_MM):
                nc.tensor.matmul(wu_p[:], wu_w[:], wu_v[:], start=True, stop=True)

            import contextlib

            loop_cm = tc.For_i(0, loop, 1) if loop else contextlib.nullcontext()
            with loop_cm:
             for _rep in range(repeat):
              # ---- phase 1: l1 = wx@x + wh@h + b0, bf16 hi/lo, exact gate.
              # L1M[m]: m-major PSUM [128 rows of half m, 256 batch].
              L1M = [ps.tile([128, 256], f32, tag=f"l1{m}", name=f"l1{m}") for m in (0, 1)]
              nmm = {0: 0, 1: 0}

              def l1_mm(m, lhsT, rhs, last=False):
                  nc.tensor.matmul(
                      L1M[m][:], lhsT, rhs, start=(nmm[m] == 0), stop=last
                  )
                  nmm[m] += 1

              # k-arrival order: k0 terms (both m), k1 terms, wx, bias;
              # m0's tail first so l1[m0] closes early.
              for k in (0, 1):
                  for m in (0, 1):
                      w_hi = HW[k][:, m * 128 : (m + 1) * 128]
                      w_lo = HW[k][:, 256 + m * 128 : 256 + (m + 1) * 128]
                      h_hi, h_lo = HW[k][:, 512:768], HW[k][:, 768:1024]
                      l1_mm(m, w_hi, h_hi)
                      l1_mm(m, w_hi, h_lo)
                      l1_mm(m, w_lo, h_hi)
              for m in (0, 1):
                  wx_hi = XW[:, m * 128 : (m + 1) * 128]
                  wx_lo = XW[:, 256 + m * 128 : 256 + (m + 1) * 128]
                  x_hi, x_lo = XW[:, 512:768], XW[:, 768:1024]
                  l1_mm(m, wx_hi, x_hi)
                  l1_mm(m, wx_hi, x_lo)
                  l1_mm(m, wx_lo, x_hi)
                  # += b0[m] (outer product with ones), closes the group
                  l1_mm(m, BIA[:, m * 128 : (m + 1) * 128], BIA[:, 256:512], last=True)

              # ---- phase 3a: u = wx@xdot (PSUM tag shared with loop W) ----
              US = [ps.tile([128, 256], f32, tag=f"w{s}", name=f"u{s}") for s in (0, 1)]
              for s in (0, 1):
                  for m in (0, 1):
                      nc.tensor.matmul(
                          US[s][:, m * 128 : (m + 1) * 128],
                          XW[:, m * 128 : (m + 1) * 128],
                          XDT[:, s * 128 : (s + 1) * 128],
                          start=True, stop=True,
                      )

              # ---- relu on DVE (PSUM read): relu[k]=relu[m] m-major bf16 ----
              relu = [apool.tile([128, 256], bf16, tag=f"relu{k}", name=f"relu{k}") for k in (0, 1)]
              for m in (0, 1):
                  nc.vector.tensor_scalar_max(relu[m][:], L1M[m][:], 0.0)

              # ---- gate mask ds = (relu > 0) = (l1 > 0), stream-major SBUF.
              # Derived from relu so the data dep orders it after relu.
              ds = [apool.tile([128, 256], bf16, tag=f"ds{s}", name=f"ds{s}") for s in (0, 1)]
              for s in (0, 1):
                  for m in (0, 1):
                      nc.vector.tensor_scalar(
                          ds[s][:, m * 128 : (m + 1) * 128],
                          relu[m][:, s * 128 : (s + 1) * 128],
                          0.0, None, _ALU.is_gt,
                      )

              # ---- phase 2: lout mms (separate tiles so tanh0 starts at
              # LO[0] completion, not the whole-bank write) ----
              LO = [ps.tile([128, 256], f32, tag=f"lo{m}", name=f"lo{m}") for m in (0, 1)]
              for m in (0, 1):
                  for k in (0, 1):
                      nc.tensor.matmul(
                          LO[m][:], WOT[:, k * 256 + m * 128 : k * 256 + (m + 1) * 128],
                          relu[k][:], start=(k == 0), stop=(k == 1),
                      )
              tanh_t = [apool.tile([128, 256], f32, tag=f"tanh{m}", name=f"tanh{m}") for m in (0, 1)]
              for m in (0, 1):
                  nc.scalar.activation(
                      tanh_t[m][:], LO[m][:], _ACT.Tanh, bias=CST[:, 4 + m : 5 + m]
                  )

              # ---- phase 3b: g = ds * u on DVE (stream-major) ----
              gs = [apool.tile([128, 256], bf16, tag=f"g{s}", name=f"g{s}") for s in (0, 1)]
              for s in (0, 1):
                  nc.vector.tensor_mul(gs[s][:], ds[s][:], US[s][:])

              # ---- dtanh, stream-major f32, on Pool; m0 ops first so dts
              # completes in stream order right after tanh1 ----
              sqs = [apool.tile([128, 256], f32, tag=f"sq{s}", name=f"sq{s}") for s in (0, 1)]
              dts = [apool.tile([128, 256], f32, tag=f"dt{s}", name=f"dt{s}") for s in (0, 1)]
              for m in (0, 1):
                  for s in (0, 1):
                      tslice = tanh_t[m][:, s * 128 : (s + 1) * 128]
                      nc.gpsimd.tensor_mul(
                          sqs[s][:, m * 128 : (m + 1) * 128], tslice, tslice
                      )
                      nc.gpsimd.tensor_scalar(
                          dts[s][:, m * 128 : (m + 1) * 128],
                          sqs[s][:, m * 128 : (m + 1) * 128],
                          -1.0, 1.0, _ALU.mult, _ALU.add,
                      )

              hdot = [apool.tile([128, 256], f32, tag=f"hdot{s}", name=f"hdot{s}") for s in (0, 1)]
              cur = {}
              cur_tg = {}

              def stage_wo_mul(s, it):
                  """wout mms (rhs = tg or g), ACT-evacuate PSUM, Pool mul."""
                  W = ps.tile([128, 256], f32, tag=f"w{s}", name=f"W{s}_{it}")
                  src = gs[s] if it == 0 else cur_tg[s]
                  for m in (0, 1):
                      for k in (0, 1):
                          nc.tensor.matmul(
                              W[:, m * 128 : (m + 1) * 128],
                              WOT[:, k * 256 + m * 128 : k * 256 + (m + 1) * 128],
                              src[:, k * 128 : (k + 1) * 128],
                              start=(k == 0), stop=(k == 1),
                          )
                  c = rot.tile([128, 256], bf16, tag=f"c{s}", name=f"c{s}")
                  nc.vector.tensor_mul(c[:], dts[s][:], W[:])
                  return c

              def stage_wh_gate(s, it):
                  """wh mms on cur[s], then tg = (l1 > 0) * V on DVE."""
                  V = ps.tile([128, 256], f32, tag=f"v{s}", name=f"V{s}_{it}")
                  for m in (0, 1):
                      for k in (0, 1):
                          nc.tensor.matmul(
                              V[:, m * 128 : (m + 1) * 128],
                              HW[k][:, m * 128 : (m + 1) * 128],
                              cur[s][:, k * 128 : (k + 1) * 128],
                              start=(k == 0), stop=(k == 1),
                          )
                  t = rot.tile([128, 256], bf16, tag=f"tg{s}", name=f"tg{s}")
                  nc.vector.tensor_mul(t[:], ds[s][:], V[:])
                  return t

              # phase 3c: jx per stream; hdot initialized from it on DVE
              for s in (0, 1):
                  c0 = stage_wo_mul(s, 0)
                  cur[s] = c0
                  nc.gpsimd.tensor_copy(hdot[s][:], c0[:])

              for it in range(1, K_TERMS + 1):
                  for s in (0, 1):
                      cur_tg[s] = stage_wh_gate(s, it)
                  for s in (0, 1):
                      newc = stage_wo_mul(s, it)
                      nc.gpsimd.tensor_add(hdot[s][:], hdot[s][:], newc[:])
                      cur[s] = newc

              # ---- output: 4 DMAs, queues alternated per m-half ----
              for s in (0, 1):
                  for m in (0, 1):
                      q = nc.sync if m == 0 else nc.scalar
                      q.dma_start(
                          hdT[m * 128 : (m + 1) * 128, s * 128 : (s + 1) * 128],
                          hdot[s][:, m * 128 : (m + 1) * 128],
                      )

    nc.compile()
    return nc


_NC = {}


def _get_nc(repeat=1, loop=0):
    key = (repeat, loop)
    if key not in _NC:
        _NC[key] = _build(repeat, loop)
    return _NC[key]


def _bf(a):
    """Round f32 -> bf16 (RNE) on host, returns ml_dtypes.bfloat16 array."""
    import ml_dtypes

    u = np.ascontiguousarray(np.asarray(a, np.float32)).view(np.uint32)
    rounded = u + np.uint32(0x7FFF) + ((u >> np.uint32(16)) & np.uint32(1))
    return (rounded >> np.uint32(16)).astype(np.uint16).view(ml_dtypes.bfloat16)


def _hilo(a):
    a = np.asarray(a, np.float32)
    hi = _bf(a)
    lo = _bf(a - hi.astype(np.float32))
    return hi, lo


def make_in_maps(h, x, xdot, wx, wh, wout, b0, b1):
    h = np.asarray(h, np.float32)
    x = np.asarray(x, np.float32)
    xdot = np.asarray(xdot, np.float32)
    wx = np.asarray(wx, np.float32)
    wh = np.asarray(wh, np.float32)
    wout = np.asarray(wout, np.float32)
    b0 = np.asarray(b0, np.float32)
    b1 = np.asarray(b1, np.float32)

    whT = np.ascontiguousarray(wh.T)  # [j, i]
    woT = np.ascontiguousarray(wout.T)
    wxT = np.ascontiguousarray(wx.T)  # [IN, H]
    wh_hi, wh_lo = _hilo(whT)
    wx_hi, wx_lo = _hilo(wxT)
    wo_hi = _bf(woT)

    d_wo = np.ascontiguousarray(
        np.concatenate([wo_hi[0:128, :], wo_hi[128:256, :]], axis=1)
    )
    bias = np.zeros((1, 512), np.float32)
    bias[0, 0:256] = b0
    bias[0, 256:512] = 1.0
    d_bias = _bf(bias)
    cst = np.stack(
        [-b0[:128], -b0[128:], b0[:128], b0[128:], b1[:128], b1[128:]], axis=1
    ).astype(np.float32)
    cst = np.ascontiguousarray(cst)

    in_maps = []
    for i in range(N_CORES):
        sl = slice(i * BL, (i + 1) * BL)
        hT = np.ascontiguousarray(h[sl].T)  # [H, BL]
        xT = np.ascontiguousarray(x[sl].T)  # [IN, BL]
        xdT = np.ascontiguousarray(xdot[sl].T)
        h_hi, h_lo = _hilo(hT)
        x_hi, x_lo = _hilo(xT)
        m = {
            "hwk0": np.ascontiguousarray(np.concatenate(
                [wh_hi[0:128], wh_lo[0:128], h_hi[0:128], h_lo[0:128]], axis=1)),
            "hwk1": np.ascontiguousarray(np.concatenate(
                [wh_hi[128:256], wh_lo[128:256], h_hi[128:256], h_lo[128:256]], axis=1)),
            "xw": np.ascontiguousarray(
                np.concatenate([wx_hi, wx_lo, x_hi, x_lo], axis=1)),
            "wo": d_wo,
            "xd": _bf(xdT),
            "bias": d_bias,
            "cst": cst,
        }
        in_maps.append(m)
    return in_maps


def kernel(h, x, xdot, wx, wh, wout, b0, b1):
    in_maps = make_in_maps(h, x, xdot, wx, wh, wout, b0, b1)
    res = run_bass_kernel_spmd(_get_nc(), in_maps, core_ids=list(range(N_CORES)))
    out = np.empty((B, H), np.float32)
    for i in range(N_CORES):
        out[i * BL : (i + 1) * BL] = res.results[i]["hdT"].T
    return out
